# revision 41
# baseline (speedup 1.0000x reference)
import os
import numpy as np
import ml_dtypes
from contextlib import ExitStack
import concourse.bass as bass
import concourse.mybir as mybir
from concourse.ap import AP as APc
from concourse.bass_utils import run_bass_kernel_spmd

B, H, W = 8, 160, 256
C, K = 700, 250
NB = 750
NH, NJ = 30, 60
NJP = 64          # NJ padded for DoubleRow weight step%16
NF = 90
TO = 500          # loss time bins (gen_sig[:, :-1])
P = H * W         # 40960
MAGIC = 400.0 / 750.0

GB, GC = 4, 2     # batch groups x cell groups = 8 cores
BL = B // GB      # 2 batches per core
CL = C // GC      # 350 cells per core
PT = 128
NPT = P // PT     # 320
QPC = 16          # pixel tiles per DMA chunk
NCH = NPT // QPC  # 40
CHW_F = QPC * CL  # 2800
CHW_J = QPC * NJP # 512
CHW = CHW_F + BL * CHW_J  # 3824 fused ft+jt chunk cols
CT_OFF = [0, 128, 256]
CT_SZ = [128, 128, CL - 256]
NPAIR = K // 2    # 125 tap pairs per diag set
FSC = 64.0        # host scale on spatial filters (fp8 subnormal dodge)
TSC = 8.0         # host scale on timecourse filters
F32 = mybir.dt.float32
BF16 = mybir.dt.bfloat16
FP8 = mybir.dt.float8e4
BF = ml_dtypes.bfloat16
F8 = ml_dtypes.float8_e4m3fn
DR = mybir.MatmulPerfMode.DoubleRow


def _build_nc():
    CPY = mybir.ActivationFunctionType.Copy
    EXP = mybir.ActivationFunctionType.Exp
    MU = mybir.AluOpType.mult
    AD = mybir.AluOpType.add
    nc = bass.Bass()
    fjc = nc.dram_tensor("fjc", (NCH, PT, CHW), FP8, kind="ExternalInput")
    hist = nc.dram_tensor("hist", (BL, NH, CL), BF16, kind="ExternalInput")
    Mmat = nc.dram_tensor("Mmat", (BL, NF, NB), BF16, kind="ExternalInput")
    fbp = nc.dram_tensor("fbp", (128, 2 * 3, TO), F32, kind="ExternalInput")
    spnp = nc.dram_tensor("spnp", (128, 2 * 3, TO), F32, kind="ExternalInput")
    mvbp = nc.dram_tensor("mvbp", (128, BL, TO), F32, kind="ExternalInput")
    tfilt = nc.dram_tensor("tfilt", (CL, K), F32, kind="ExternalInput")
    ident = nc.dram_tensor("ident", (128, 128), BF16, kind="ExternalInput")
    part = nc.dram_tensor("part", (BL, CL), F32, kind="ExternalOutput")

    N_PRO = 10
    es = ExitStack()
    with es:
        ident_sb = es.enter_context(nc.sbuf_tensor("ident_sb", [128, 128], BF16))
        tf0 = es.enter_context(nc.sbuf_tensor("tf0", [128, K], F32))
        tf1 = es.enter_context(nc.sbuf_tensor("tf1", [128, K], F32))
        tf2 = es.enter_context(nc.sbuf_tensor("tf2", [CT_SZ[2], K], F32))
        hist0 = es.enter_context(nc.sbuf_tensor("hist0", [NH, CL], BF16))
        hist1 = es.enter_context(nc.sbuf_tensor("hist1", [NH, CL], BF16))
        mh0 = es.enter_context(nc.sbuf_tensor("mh0", [NH, NB], BF16))
        mh1 = es.enter_context(nc.sbuf_tensor("mh1", [NH, NB], BF16))
        mj0 = es.enter_context(nc.sbuf_tensor("mj0", [NJ, NB], BF16))
        mj1 = es.enter_context(nc.sbuf_tensor("mj1", [NJ, NB], BF16))
        mvb_sb = es.enter_context(nc.sbuf_tensor("mvb_sb", [128, BL, TO], F32))
        fb_sb = es.enter_context(nc.sbuf_tensor("fb_sb", [128, 6, TO], F32))
        spn_sb = es.enter_context(nc.sbuf_tensor("spn_sb", [128, 6, TO], F32))
        fj_sb = es.enter_context(nc.sbuf_tensor("fj_sb", [PT, 4, CHW], FP8))
        spatj0 = es.enter_context(nc.sbuf_tensor("spatj0", [NJ, CL], BF16))
        spatj1 = es.enter_context(nc.sbuf_tensor("spatj1", [NJ, CL], BF16))
        up_sb = es.enter_context(nc.sbuf_tensor("up_sb", [128, 4, 2, NB], FP8))
        dg_sb = es.enter_context(nc.sbuf_tensor("dg_sb", [128, 3, NPAIR, 2, 128], FP8))
        gen_sb = es.enter_context(nc.sbuf_tensor("gen_sb", [128, BL, TO], F32))
        tmpa = es.enter_context(nc.sbuf_tensor("tmpa", [128, BL, TO], F32))
        tmpb = es.enter_context(nc.sbuf_tensor("tmpb", [128, BL, TO], F32))
        junk_sb = es.enter_context(nc.sbuf_tensor("junk_sb", [128, 4, TO], F32))
        r1_sb = es.enter_context(nc.sbuf_tensor("r1_sb", [128, 10], F32))
        r2_sb = es.enter_context(nc.sbuf_tensor("r2_sb", [128, 10], F32))
        res_sb = es.enter_context(nc.sbuf_tensor("res_sb", [128, 6], F32))
        spat_ps0 = es.enter_context(nc.psum_tensor("spat_ps0", [NJ, CL], F32))
        spat_ps1 = es.enter_context(nc.psum_tensor("spat_ps1", [NJ, CL], F32))
        up_ps = es.enter_context(nc.psum_tensor("up_ps", [128, 4, 512], F32))
        acc_ps = es.enter_context(nc.psum_tensor("acc_ps", [128, 2, 512], F32))
        prosem = es.enter_context(nc.semaphore("prosem"))
        dsm = [es.enter_context(nc.semaphore(f"dsm{i}")) for i in range(4)]
        psem = es.enter_context(nc.semaphore("psem"))
        scsem = es.enter_context(nc.semaphore("scsem"))
        upsem = es.enter_context(nc.semaphore("upsem"))
        ucsem = es.enter_context(nc.semaphore("ucsem"))
        dgsem0 = es.enter_context(nc.semaphore("dgsem0"))
        dgs0b = es.enter_context(nc.semaphore("dgs0b"))
        dgsem1 = es.enter_context(nc.semaphore("dgsem1"))
        dgsem2 = es.enter_context(nc.semaphore("dgsem2"))
        chsem = es.enter_context(nc.semaphore("chsem"))
        gsem = es.enter_context(nc.semaphore("gsem"))
        tsem = es.enter_context(nc.semaphore("tsem"))
        asem = es.enter_context(nc.semaphore("asem"))
        latsem = es.enter_context(nc.semaphore("latsem"))
        rsem = es.enter_context(nc.semaphore("rsem"))
        osem = es.enter_context(nc.semaphore("osem"))
        block = es.enter_context(nc.Block())

        hist_sb = [hist0, hist1]
        mh_sb = [mh0, mh1]
        mj_sb = [mj0, mj1]
        spatj_sb = [spatj0, spatj1]
        spat_ps = [spat_ps0, spat_ps1]
        tf_sb = [tf0, tf1, tf2]
        dgsem = [dgsem0, dgsem1, dgsem2]


        @block.sync
        def _(sync):
            for ch in range(4):
                sync.dma_start(fj_sb[:, ch], fjc[ch]).then_inc(dsm[ch], 16)
            # spatial streaming, 4-slot buffered fused chunks
            for ch in range(4, NCH):
                sync.wait_ge(psem, ch - 3)
                o = ch % 4
                sync.dma_start(fj_sb[:, o], fjc[ch]).then_inc(dsm[o], 16)
            sync.dma_start(mvb_sb[:], mvbp[:]).then_inc(latsem, 16)
            sync.dma_start(fb_sb[:], fbp[:]).then_inc(latsem, 16)
            sync.dma_start(spn_sb[:], spnp[:]).then_inc(latsem, 16)
            # outputs
            for ct in range(3):
                mc = CT_SZ[ct]
                sync.wait_ge(rsem, ct + 1)
                for b in range(BL):
                    j = 2 * ct + b
                    sync.dma_start(part[b, CT_OFF[ct]:CT_OFF[ct] + mc],
                                   res_sb[0:mc, j:j + 1]).then_inc(osem, 16)
            sync.wait_ge(osem, 16 * 6)

        @block.tensor
        def _(tensor):
            fj_h = fj_sb[0:PT, 0, 0:1]
            ROW = 4 * CHW

            def ft_ap(o, q):
                return APc(fj_h.tensor, o * CHW + q * CL,
                           [[ROW, PT], [CL, 2], [1, CL]])

            def jt_ap(o, b, q):
                return APc(fj_h.tensor, o * CHW + CHW_F + b * CHW_J + q * NJP,
                           [[ROW, PT], [NJP, 2], [1, NJ]])

            for ch in range(NCH):
                o = ch % 4
                tensor.wait_ge(dsm[o], 16 * (ch // 4 + 1))
                mm = None
                for q in range(0, QPC, 2):
                    for b in range(BL):
                        mm = tensor.matmul(spat_ps[b][:, :],
                                           jt_ap(o, b, q),
                                           ft_ap(o, q),
                                           start=(ch == 0 and q == 0),
                                           stop=(ch == NCH - 1 and q == QPC - 2),
                                           skip_group_check=True,
                                           perf_mode=DR)
                mm.then_inc(psem, 1)

            tensor.wait_ge(prosem, 16 * N_PRO)
            tensor.wait_ge(scsem, 1)

            def emit_upsample(ct):
                mc, off = CT_SZ[ct], CT_OFF[ct]
                mm = None
                for b in range(BL):
                    for ch2 in range(2):
                        tensor.matmul(up_ps[0:mc, 2 * b + ch2, 0:375],
                                      hist_sb[b][:, off:off + mc],
                                      mh_sb[b][:, ch2 * 375:(ch2 + 1) * 375],
                                      start=True, stop=False, skip_group_check=True)
                        mm = tensor.matmul(up_ps[0:mc, 2 * b + ch2, 0:375],
                                           spatj_sb[b][:, off:off + mc],
                                           mj_sb[b][:, ch2 * 375:(ch2 + 1) * 375],
                                           start=False, stop=True, skip_group_check=True)
                mm.then_inc(upsem, 1)

            emit_upsample(0)
            for ct in range(3):
                mc = CT_SZ[ct]
                us = ct % 2
                if ct + 1 < 3:
                    tensor.wait_ge(ucsem, ct + 1)   # up_ps free again
                    emit_upsample(ct + 1)
                tensor.wait_ge(ucsem, ct + 1)       # up_sb[us] ready
                tensor.wait_ge(dgsem[ct], 1)
                halves = (((None, 0, 500),) if ct < 2 else
                          ((0, 250, 250), (None, 0, 125), (2, 125, 125)))
                for L, c0, cw in halves:
                    if ct >= 1 and L is None and c0 == 0:
                        tensor.wait_ge(gsem, ct)    # acc_ps consumed
                    for p in range(NPAIR):
                        if ct == 0 and p == NPAIR // 2:
                            tensor.wait_ge(dgs0b, 1)
                        for b in range(BL):
                            if L is not None:
                                ops = up_ps[0:mc, L + b, 0:cw]
                            else:
                                ops = acc_ps[0:mc, b, c0:c0 + cw]
                            mm = tensor.matmul(ops,
                                               dg_sb[0:mc, ct, p, 0:2, 0:mc],
                                               up_sb[0:mc, 2 * us + b, 0:2,
                                                     2 * p + c0:2 * p + c0 + cw],
                                               start=(p == 0), stop=(p == NPAIR - 1),
                                               skip_group_check=True,
                                               perf_mode=DR)
                    mm.then_inc(chsem, 1)

        @block.vector
        def _(vector):
            vector.wait_ge(prosem, 16 * N_PRO)
            mm = None
            for p in range(NPAIR // 2, NPAIR):
                for ko in range(2):
                    col = 2 * p + ko
                    mm = vector.tensor_scalar_mul(dg_sb[0:128, 0, p, ko, 0:128],
                                                  ident_sb[0:128, 0:128],
                                                  tf0[0:128, col:col + 1])
            mm.then_inc(dgs0b, 1)
            mm = None
            for p in range(NPAIR):
                for ko in range(2):
                    col = 2 * p + ko
                    mm = vector.tensor_scalar_mul(dg_sb[0:128, 1, p, ko, 0:128],
                                                  ident_sb[0:128, 0:128],
                                                  tf1[0:128, col:col + 1])
            mm.then_inc(dgsem1, 1)
            vector.wait_ge(latsem, 48)
            ng = 0
            nt = 0
            for ct in range(3):
                mc = CT_SZ[ct]
                halves = (((None, 0, 500),) if ct < 2 else
                          ((0, 250, 250), (None, 0, 125), (2, 125, 125)))
                for hi, (L, c0, cw) in enumerate(halves):
                    ng += 1
                    nt += 2
                    vector.wait_ge(chsem, ng)         # conv (half-)tile done
                    if ct >= 1 and hi == 0:
                        vector.wait_ge(tsem, 2 * ct)  # drain own tmpb read of prev tile
                    acc_src = (up_ps[0:mc, L:L + 2, 0:cw] if L is not None else
                               acc_ps[0:mc, 0:2, c0:c0 + cw])
                    vector.scalar_tensor_tensor(gen_sb[0:mc, 0:2, c0:c0 + cw],
                                                acc_src,
                                                1.0 / TSC,
                                                fb_sb[0:mc, 2 * ct:2 * ct + 2,
                                                      c0:c0 + cw],
                                                MU, AD).then_inc(gsem, 1)
                    vector.wait_ge(gsem, ng)          # drain gen write
                    if hi == 0:
                        vector.wait_ge(asem, ct)      # tmpa/tmpb free
                    vector.tensor_add(tmpa[0:mc, 0:2, c0:c0 + cw],
                                      gen_sb[0:mc, 0:2, c0:c0 + cw],
                                      mvb_sb[0:mc, 0:2, c0:c0 + cw]).then_inc(tsem, 1)
                    vector.tensor_mul(tmpb[0:mc, 0:2, c0:c0 + cw],
                                      gen_sb[0:mc, 0:2, c0:c0 + cw],
                                      spn_sb[0:mc, 2 * ct:2 * ct + 2,
                                             c0:c0 + cw]).then_inc(tsem, 1)
                    if ct >= 1 and hi == 0:
                        pm = CT_SZ[ct - 1]
                        j0 = 2 * (ct - 1)
                        vector.tensor_add(res_sb[0:pm, j0:j0 + 2],
                                          r1_sb[0:pm, j0:j0 + 2],
                                          r2_sb[0:pm, j0:j0 + 2]).then_inc(rsem, 1)
            m2 = CT_SZ[2]
            vector.wait_ge(asem, 5)
            vector.tensor_add(r1_sb[0:m2, 4:6], r1_sb[0:m2, 4:6],
                              r1_sb[0:m2, 6:8]).then_inc(tsem, 1)
            vector.tensor_add(r2_sb[0:m2, 4:6], r2_sb[0:m2, 4:6],
                              r2_sb[0:m2, 6:8]).then_inc(tsem, 1)
            vector.wait_ge(tsem, 12)
            vector.tensor_add(r1_sb[0:m2, 4:6], r1_sb[0:m2, 4:6],
                              r1_sb[0:m2, 8:10]).then_inc(tsem, 1)
            vector.tensor_add(r2_sb[0:m2, 4:6], r2_sb[0:m2, 4:6],
                              r2_sb[0:m2, 8:10]).then_inc(tsem, 1)
            vector.wait_ge(tsem, 14)                  # drain own r1/r2 merges
            vector.tensor_add(res_sb[0:m2, 4:6], r1_sb[0:m2, 4:6],
                              r2_sb[0:m2, 4:6]).then_inc(rsem, 1)

        @block.gpsimd
        def _(gp):
            gp.wait_ge(prosem, 16 * N_PRO)
            mm = None
            for p in range(NPAIR // 2):
                for ko in range(2):
                    col = 2 * p + ko
                    mm = gp.tensor_scalar_mul(dg_sb[0:128, 0, p, ko, 0:128],
                                              ident_sb[0:128, 0:128],
                                              tf0[0:128, col:col + 1])
            mm.then_inc(dgsem0, 1)
            m = CT_SZ[2]
            mm = None
            for p in range(NPAIR):
                for ko in range(2):
                    col = 2 * p + ko
                    mm = gp.tensor_scalar_mul(dg_sb[0:m, 2, p, ko, 0:m],
                                              ident_sb[0:m, 0:m],
                                              tf2[0:m, col:col + 1])
            mm.then_inc(dgsem2, 1)

        @block.scalar
        def _(scalar):
            scalar.dma_start(ident_sb[:], ident[:]).then_inc(prosem, 16)
            for ct in range(3):
                scalar.dma_start(tf_sb[ct][0:CT_SZ[ct], :],
                                 tfilt[CT_OFF[ct]:CT_OFF[ct] + CT_SZ[ct], :]).then_inc(prosem, 16)
            for b in range(BL):
                scalar.dma_start(hist_sb[b][:], hist[b]).then_inc(prosem, 16)
            for b in range(BL):
                scalar.dma_start(mh_sb[b][:], Mmat[b, 0:NH]).then_inc(prosem, 16)
            for b in range(BL):
                scalar.dma_start(mj_sb[b][:], Mmat[b, NH:NF]).then_inc(prosem, 16)
            scalar.wait_ge(prosem, 16 * N_PRO)
            scalar.wait_ge(psem, NCH)
            scalar.mul(spatj_sb[0][:, :], spat_ps0[:, :], 1.0 / FSC)
            scalar.mul(spatj_sb[1][:, :], spat_ps1[:, :], 1.0 / FSC).then_inc(scsem, 1)

            def up_copy(ct):
                mc = CT_SZ[ct]
                us = ct % 2
                bs = (0, 1)
                scalar.wait_ge(upsem, ct + 1)
                if ct >= 2:
                    scalar.wait_ge(chsem, ct - 1)  # conv(ct-2) done with slot
                mm = None
                for b in bs:
                    lane = 2 * us + b
                    scalar.activation(up_sb[0:mc, lane, 0, 0:750],
                                      up_ps[0:mc, 2 * b:2 * b + 2, 0:375], CPY)
                    # r=1 plane: up shifted left by one bin
                    scalar.activation(up_sb[0:mc, lane, 1, 0:374],
                                      up_ps[0:mc, 2 * b, 1:375], CPY)
                    mm = scalar.activation(up_sb[0:mc, lane, 1, 374:749],
                                           up_ps[0:mc, 2 * b + 1, 0:375], CPY)
                mm.then_inc(ucsem, 1)

            up_copy(0)
            up_copy(1)
            nt = 0
            na = 0
            for ct in range(3):
                mc = CT_SZ[ct]
                halves = (((None, 0, 500),) if ct < 2 else
                          ((0, 250, 250), (None, 0, 125), (2, 125, 125)))
                for hi, (L, c0, cw) in enumerate(halves):
                    jc = 2 * ct + 2 * hi
                    nt += 2
                    na += 1
                    scalar.wait_ge(tsem, nt - 1)
                    if na >= 2:
                        scalar.wait_ge(asem, na - 1)  # drain own junk writes
                    scalar.activation(junk_sb[0:mc, 0, 0:cw], tmpa[0:mc, 0, c0:c0 + cw],
                                      EXP, accum_out=r1_sb[0:mc, jc:jc + 1])
                    scalar.activation(junk_sb[0:mc, 1, 0:cw], tmpa[0:mc, 1, c0:c0 + cw],
                                      EXP, accum_out=r1_sb[0:mc, jc + 1:jc + 2])
                    scalar.wait_ge(tsem, nt)
                    scalar.activation(junk_sb[0:mc, 2, 0:cw], tmpb[0:mc, 0, c0:c0 + cw],
                                      CPY, accum_out=r2_sb[0:mc, jc:jc + 1])
                    scalar.activation(junk_sb[0:mc, 3, 0:cw], tmpb[0:mc, 1, c0:c0 + cw],
                                      CPY, accum_out=r2_sb[0:mc, jc + 1:jc + 2]).then_inc(asem, 1)
                    if ct == 0:
                        up_copy(2)
    return nc


_NC_CACHE = {}


def _host_prep(inputs):
    img = np.asarray(inputs["batched_image"], dtype=np.float32)
    spikes = np.asarray(inputs["batched_spikes"], dtype=np.float32)
    em = np.asarray(inputs["eye_movements"]).astype(np.int64)
    tmask = np.asarray(inputs["time_mask"], dtype=np.float32)
    sel = np.asarray(inputs["forward_sel"]).astype(np.int64)
    fw = np.asarray(inputs["forward_weights"], dtype=np.float32)
    F = np.asarray(inputs["stacked_flat_spat_filters"], dtype=np.float32)
    tcf = np.asarray(inputs["stacked_timecourse_filters"], dtype=np.float32)
    fbg = np.asarray(inputs["precomputed_feedback_gensig"], dtype=np.float32)
    histf = np.asarray(inputs["precomputed_history_frames"], dtype=np.float32)

    # jitter on host (pure gather, exact)
    jit = np.zeros((B, NJ, H, W), dtype=np.float32)
    for b in range(B):
        for f in range(NJ):
            dy, dx = int(em[b, f, 0]), int(em[b, f, 1])
            ys, xs = max(0, -dy), max(0, -dx)
            ye, xe = min(H, H - dy), min(W, W - dx)
            if ye > ys and xe > xs:
                jit[b, f, ys:ye, xs:xe] = img[b, ys + dy:ye + dy, xs + dx:xe + dx]
    jitT = jit.reshape(B, NJ, P).transpose(0, 2, 1)                 # (B,P,NJ)
    jitp = np.zeros((B, NCH, PT, QPC, NJP), dtype=np.float32)
    jitp[..., 0:NJ] = jitT.reshape(B, NCH, QPC, PT, NJ).transpose(0, 1, 3, 2, 4)
    jitc = np.ascontiguousarray(jitp.reshape(B, NCH, PT, CHW_J)).astype(F8)
    jit2_h = [np.stack([jitc[BL * bg + b] for b in range(BL)], axis=2)
              .reshape(NCH, PT, BL * CHW_J) for bg in range(GB)]

    FTf = F.T * np.float32(FSC)                                     # (P,C)
    ftc_h = []
    for cg in range(GC):
        X = FTf[:, cg * CL:(cg + 1) * CL]
        ftc_h.append(np.ascontiguousarray(
            X.reshape(NCH, QPC, PT, CL).transpose(0, 2, 1, 3)
            .reshape(NCH, PT, CHW_F)).astype(F8))

    # upsample mixing matrix M[f,t]
    Mm = np.zeros((B, NF, NB), dtype=np.float32)
    tix = np.arange(NB)
    for b in range(B):
        np.add.at(Mm[b], (sel[b, :, 0], tix), fw[b, :, 0])
        np.add.at(Mm[b], (sel[b, :, 1], tix), fw[b, :, 1])
    Mmb = Mm.astype(BF)

    mv = tmask * np.float32(MAGIC)                                  # (B,500)
    with np.errstate(divide="ignore"):
        lmv = np.log(mv).astype(np.float32)
    spn_all = -(spikes[:, :, K:] * mv[:, None, :])                  # (B,C,500)
    fb5 = fbg[:, :, :TO]
    histb = histf.astype(BF)                                        # (B,NH,C)
    identity = np.eye(128, dtype=np.float32).astype(BF)
    tcf_s = tcf * np.float32(TSC)

    in_maps = []
    for i in range(8):
        bg, cg = i // GC, i % GC
        bs = slice(BL * bg, BL * (bg + 1))
        cs = slice(CL * cg, CL * (cg + 1))
        fbp = np.zeros((128, 6, TO), dtype=np.float32)
        spnp = np.zeros((128, 6, TO), dtype=np.float32)
        for ct in range(3):
            mc, off = CT_SZ[ct], CT_OFF[ct]
            for b in range(BL):
                fbp[0:mc, 2 * ct + b, :] = fb5[BL * bg + b, cg * CL + off:cg * CL + off + mc, :]
                spnp[0:mc, 2 * ct + b, :] = spn_all[BL * bg + b, cg * CL + off:cg * CL + off + mc, :]
        mvbp = np.ascontiguousarray(np.broadcast_to(
            lmv[bs][None, :, :], (128, BL, TO)))
        in_maps.append({
            "fjc": np.ascontiguousarray(
                np.concatenate([ftc_h[cg], jit2_h[bg]], axis=2)),
            "hist": np.ascontiguousarray(histb[bs][:, :, cs]),
            "Mmat": Mmb[bs],
            "tfilt": np.ascontiguousarray(tcf_s[cs]),
            "fbp": fbp,
            "spnp": spnp,
            "mvbp": mvbp,
            "ident": identity,
        })
    return in_maps


def kernel(**inputs) -> np.ndarray:
    in_maps = _host_prep(inputs)
    if "nc" not in _NC_CACHE:
        _NC_CACHE["nc"] = _build_nc()
    nc = _NC_CACHE["nc"]

    if os.environ.get("KTRACE"):
        res = run_bass_kernel_spmd(
            nc, in_maps, core_ids=list(range(8)), trace=True,
            trace_cores=[0], tmpdir=os.environ.get("KTRACE_DIR") or None)
        kernel.last_results = res
    else:
        res = run_bass_kernel_spmd(nc, in_maps, core_ids=list(range(8)))
    out = np.zeros(B, dtype=np.float64)
    for i in range(8):
        bg = i // GC
        out[BL * bg:BL * (bg + 1)] += res.results[i]["part"].sum(axis=1, dtype=np.float64)
    return out.astype(np.float32)
